# revision 1
# baseline (speedup 1.0000x reference)
"""Gaussian (norm) kernel matrix on 8 Trainium2 NeuronCores.

out[b, p] = exp(-sqrt(||x_b - proto_p||^2) / (2*sigma^2)), sigma = 1.

Sharding: x split along batch across 8 cores (1024 rows each); prototypes
replicated. Each core computes its [1024, 8192] slice.

Per-core math: d2 = x_sq + p_sq - 2*x.p is produced directly in PSUM by an
augmented matmul: K=256 cross term in bf16 (two K=128 accumulating matmuls
over (-2*p)^T) plus one K=4 bf16 matmul whose rows carry hi/lo bf16 splits
of p_sq and x_sq against ones (hi/lo restores ~fp32 precision of the norm
terms). ScalarE then does dist = sqrt(PSUM) and out = exp(-0.5*dist + 10*ln2)
written as fp16 (the 2^10 pre-scale keeps the tiny outputs in fp16 normal
range; the host divides it back out exactly). sqrt/exp live in different ACT
table sets (~2.7us per switch), so b-tiles run in two phases of 4 and the
ACT instruction order is pinned with scheduling-only deps — without the pin
the Tile scheduler interleaves sqrt/exp and quadruples the table loads.
"""

import numpy as np

import concourse.bacc as bacc
import concourse.mybir as mybir
import concourse.tile as tile
from concourse.tile import add_dep_helper
from concourse.bass_utils import run_bass_kernel_spmd

N_CORES = 8
B, P, F = 8192, 8192, 256
BS = B // N_CORES  # 1024 batch rows per core
BT = 128  # batch tile (partition dim)
NB = BS // BT  # 8 batch tiles per core
NCHUNK = 2048  # PSUM tile free size (4 banks; 2 bufs = all 8)
PHASE = 4  # b-tiles per activation-table phase
OUT_SCALE_LOG2 = 10  # exp pre-scale: out16 = 2^10 * exp(-dist/2)
F32 = mybir.dt.float32
F16 = mybir.dt.float16
BF16 = mybir.dt.bfloat16
AF = mybir.ActivationFunctionType


def build_nc(n_iters: int = 1):
    nc = bacc.Bacc("TRN2", target_bir_lowering=False, debug=False,
                   num_devices=N_CORES)
    xT_d = nc.dram_tensor("xT", [2, BT, BS], BF16, kind="ExternalInput")
    pT_d = nc.dram_tensor("pT", [2, BT, P], BF16, kind="ExternalInput")
    augL_d = nc.dram_tensor("augL", [4, BS], BF16, kind="ExternalInput")
    augR_d = nc.dram_tensor("augR", [4, P], BF16, kind="ExternalInput")
    out_d = nc.dram_tensor("out", [BS, P], F16, kind="ExternalOutput")

    with tile.TileContext(nc) as tc:
        with (
            tc.tile_pool(name="const", bufs=1) as cpool,
            tc.tile_pool(name="dist", bufs=PHASE) as dpool,
            tc.tile_pool(name="o16", bufs=2) as opool,
            tc.tile_pool(name="psum", bufs=2, space="PSUM") as ppool,
        ):
            xT = [cpool.tile([BT, BS], BF16, name=f"xT{k}", tag=f"xT{k}")
                  for k in range(2)]
            pT = [cpool.tile([BT, P], BF16, name=f"pT{k}", tag=f"pT{k}")
                  for k in range(2)]
            augL = cpool.tile([4, BS], BF16)
            augR = cpool.tile([4, P], BF16)
            ebias = cpool.tile([BT, 1], F32)
            nc.gpsimd.memset(ebias[:], float(OUT_SCALE_LOG2 * np.log(2.0)))
            for k in range(2):
                nc.sync.dma_start(xT[k][:], xT_d[k])
            nc.sync.dma_start(augL[:], augL_d[:])
            nc.sync.dma_start(augR[:], augR_d[:])
            for k in range(2):
                nc.sync.dma_start(pT[k][:], pT_d[k])

            acts = []  # pinned ACT execution order
            for _ in range(n_iters):
                for p0 in range(0, NB, PHASE):
                    bts = range(p0, min(p0 + PHASE, NB))
                    dists = {}
                    for bt in bts:
                        dist = dpool.tile([BT, P], F32)
                        dists[bt] = dist
                        bsl = slice(bt * BT, (bt + 1) * BT)
                        for c0 in range(0, P, NCHUNK):
                            ps = ppool.tile([BT, NCHUNK], F32)
                            for j in range(0, NCHUNK, 512):
                                n = slice(c0 + j, c0 + j + 512)
                                o = ps[:, j:j + 512]
                                nc.tensor.matmul(o, xT[0][:, bsl], pT[0][:, n],
                                                 start=True, stop=False)
                                nc.tensor.matmul(o, xT[1][:, bsl], pT[1][:, n],
                                                 start=False, stop=False)
                                nc.tensor.matmul(o, augL[:, bsl], augR[:, n],
                                                 start=False, stop=True)
                            acts.append(nc.scalar.activation(
                                dist[:, c0:c0 + NCHUNK], ps[:], AF.Sqrt))
                    for bt in bts:
                        for h0 in range(0, P, P // 2):
                            o16 = opool.tile([BT, P // 2], F16)
                            acts.append(nc.scalar.activation(
                                o16[:], dists[bt][:, h0:h0 + P // 2],
                                AF.Exp, scale=-0.5, bias=ebias[:]))
                            nc.sync.dma_start(
                                out_d[bt * BT:(bt + 1) * BT, h0:h0 + P // 2],
                                o16[:])
            for a, b in zip(acts, acts[1:]):
                add_dep_helper(b.ins, a.ins, sync=False,
                               reason="pin ACT order for table-set grouping")
    nc.compile()
    return nc


def _prep_inputs(x: np.ndarray, prototypes: np.ndarray):
    """Host-side shard + layout prep. Returns per-core in_maps."""
    bf16 = mybir.dt.np(BF16)
    x = np.ascontiguousarray(x, dtype=np.float32)
    p = np.ascontiguousarray(prototypes, dtype=np.float32)

    x_sq = np.sum(x * x, axis=-1)  # [B]
    p_sq = np.sum(p * p, axis=-1)  # [P]

    def hilo(v):
        hi = v.astype(bf16)
        lo = (v - hi.astype(np.float32)).astype(bf16)
        return hi, lo

    psq_hi, psq_lo = hilo(p_sq)
    ones_p = np.ones([P], dtype=bf16)
    augR = np.stack([psq_hi, psq_lo, ones_p, ones_p])  # [4, P]

    # [2, BT, P] with pT[k, r, n] = -2 * p[n, k*128 + r]
    pT = np.ascontiguousarray((-2.0 * p).T.reshape(2, BT, P)).astype(bf16)

    in_maps = []
    for c in range(N_CORES):
        xc = x[c * BS:(c + 1) * BS]  # [BS, F]
        xT = np.ascontiguousarray(xc.T.reshape(2, BT, BS)).astype(bf16)
        xsq_hi, xsq_lo = hilo(x_sq[c * BS:(c + 1) * BS])
        ones_b = np.ones([BS], dtype=bf16)
        augL = np.stack([ones_b, ones_b, xsq_hi, xsq_lo])  # [4, BS]
        in_maps.append({"xT": xT, "pT": pT, "augL": augL, "augR": augR})
    return in_maps


def _gather(per_core_outs):
    """fp16 shards -> fp32 full output, undoing the exact 2^10 pre-scale."""
    out = np.concatenate(per_core_outs, axis=0).astype(np.float32)
    out *= np.float32(2.0 ** -OUT_SCALE_LOG2)
    return out


def kernel(x: np.ndarray, prototypes: np.ndarray) -> np.ndarray:
    nc = build_nc()
    in_maps = _prep_inputs(x, prototypes)
    res = run_bass_kernel_spmd(nc, in_maps, list(range(N_CORES)))
    return _gather([res.results[c]["out"] for c in range(N_CORES)])



# revision 2
# speedup vs baseline: 1324.5931x; 1324.5931x over previous
"""Gaussian (norm) kernel matrix on 8 Trainium2 NeuronCores.

out[b, p] = exp(-sqrt(||x_b - proto_p||^2) / (2*sigma^2)), sigma = 1.

Sharding: x split along batch across 8 cores (1024 rows each); prototypes
replicated. Each core computes its [1024, 8192] slice.

Per-core pipeline (engines decoupled so ACT, the bottleneck, never waits):
- PE: psum = -2 x.p (two K=128 bf16 matmuls) + p_sq (K=2 bf16 aug matmul
  with hi/lo split rows against ones) into [128, 2048] PSUM chunks.
- DVE: d2_16 = fp16(psum + x_sq) - drains PSUM with the per-partition x_sq
  add folded in, freeing PSUM for the PE to run ahead while ACT is busy.
- ACT: per 4-b-tile phase, one big sqrt over [128, 4*8192] then exp chunks
  written as fp16 with a 2^10 pre-scale (keeps tiny outputs in fp16 normal
  range; the host divides it back out exactly). sqrt/exp live in different
  ACT table sets (~2.7us per switch), so the ACT instruction order is pinned
  with scheduling-only deps to get exactly 2 loads per phase.
"""

import numpy as np

import concourse.bacc as bacc
import concourse.mybir as mybir
import concourse.tile as tile
from concourse.tile import add_dep_helper
from concourse.bass_utils import run_bass_kernel_spmd

N_CORES = 8
B, P, F = 8192, 8192, 256
BS = B // N_CORES  # 1024 batch rows per core
BT = 128  # batch tile (partition dim)
NB = BS // BT  # 8 batch tiles per core
NCHUNK = 2048  # PSUM tile free size (4 banks; 2 bufs = all 8)
PHASE = 4  # b-tiles per activation-table phase
OCHUNK = 4096  # exp output chunk (free dim)
OUT_SCALE_LOG2 = 10  # exp pre-scale: out16 = 2^10 * exp(-dist/2)
F32 = mybir.dt.float32
F16 = mybir.dt.float16
BF16 = mybir.dt.bfloat16
AF = mybir.ActivationFunctionType


def build_nc(n_iters: int = 1, loop: bool = False):
    nc = bacc.Bacc("TRN2", target_bir_lowering=False, debug=False,
                   num_devices=N_CORES)
    xT_d = nc.dram_tensor("xT", [2, BT, BS], BF16, kind="ExternalInput")
    pT_d = nc.dram_tensor("pT", [2, BT, P], BF16, kind="ExternalInput")
    augR_d = nc.dram_tensor("augR", [2, P], BF16, kind="ExternalInput")
    xsq_d = nc.dram_tensor("xsq", [BT, NB], F32, kind="ExternalInput")
    out_d = nc.dram_tensor("out", [BS, P], F16, kind="ExternalOutput")

    with tile.TileContext(nc) as tc:
        with (
            tc.tile_pool(name="const", bufs=1) as cpool,
            tc.tile_pool(name="d2", bufs=1) as d2pool,
            tc.tile_pool(name="dist", bufs=1) as dpool,
            tc.tile_pool(name="o16", bufs=2) as opool,
            tc.tile_pool(name="psum", bufs=2, space="PSUM") as ppool,
        ):
            xT = [cpool.tile([BT, BS], BF16, name=f"xT{k}", tag=f"xT{k}")
                  for k in range(2)]
            pT = [cpool.tile([BT, P], BF16, name=f"pT{k}", tag=f"pT{k}")
                  for k in range(2)]
            augL = cpool.tile([2, BS], BF16)
            augR = cpool.tile([2, P], BF16)
            xsq = cpool.tile([BT, NB], F32)
            ebias = cpool.tile([BT, 1], F32)
            nc.gpsimd.memset(ebias[:], float(OUT_SCALE_LOG2 * np.log(2.0)))
            nc.gpsimd.memset(augL[:], 1.0)
            for k in range(2):
                nc.sync.dma_start(xT[k][:], xT_d[k])
            nc.sync.dma_start(augR[:], augR_d[:])
            nc.sync.dma_start(xsq[:], xsq_d[:])
            for k in range(2):
                nc.sync.dma_start(pT[k][:], pT_d[k])

            d2_16 = d2pool.tile([BT, PHASE * P], F16, name="d2_16", tag="d2")
            dist16 = dpool.tile([BT, PHASE * P], F16, name="dist16",
                                tag="dist")

            acts = []  # pinned ACT execution order

            def body(_iv=None):
                for p0 in range(0, NB, PHASE):
                    bts = range(p0, min(p0 + PHASE, NB))
                    for bt in bts:
                        bsl = slice(bt * BT, (bt + 1) * BT)
                        xo = (bt - p0) * P
                        for c0 in range(0, P, NCHUNK):
                            ps = ppool.tile([BT, NCHUNK], F32)
                            for j in range(0, NCHUNK, 512):
                                n = slice(c0 + j, c0 + j + 512)
                                o = ps[:, j:j + 512]
                                nc.tensor.matmul(o, xT[0][:, bsl],
                                                 pT[0][:, n],
                                                 start=True, stop=False)
                                nc.tensor.matmul(o, xT[1][:, bsl],
                                                 pT[1][:, n],
                                                 start=False, stop=False)
                                nc.tensor.matmul(o, augL[:, bsl],
                                                 augR[:, n],
                                                 start=False, stop=True)
                            nc.vector.tensor_scalar_add(
                                d2_16[:, xo + c0:xo + c0 + NCHUNK], ps[:],
                                xsq[:, bt:bt + 1])
                    acts.append(nc.scalar.activation(
                        dist16[:], d2_16[:], AF.Sqrt))
                    for k in range(0, PHASE * P, OCHUNK):
                        bt = p0 + k // P
                        c0 = k % P
                        o16 = opool.tile([BT, OCHUNK], F16)
                        acts.append(nc.scalar.activation(
                            o16[:], dist16[:, k:k + OCHUNK],
                            AF.Exp, scale=-0.5, bias=ebias[:]))
                        nc.sync.dma_start(
                            out_d[bt * BT:(bt + 1) * BT, c0:c0 + OCHUNK],
                            o16[:])

            if loop and n_iters > 1:
                with tc.For_i(0, n_iters, 1,
                              hint_engines=(mybir.EngineType.PE,
                                            mybir.EngineType.Activation,
                                            mybir.EngineType.DVE)):
                    body()
            else:
                for _ in range(n_iters):
                    body()
            for a, b in zip(acts, acts[1:]):
                add_dep_helper(b.ins, a.ins, sync=False,
                               reason="pin ACT order for table-set grouping")
    nc.compile()
    return nc


def _prep_inputs(x: np.ndarray, prototypes: np.ndarray):
    """Host-side shard + layout prep. Returns per-core in_maps."""
    bf16 = mybir.dt.np(BF16)
    x = np.ascontiguousarray(x, dtype=np.float32)
    p = np.ascontiguousarray(prototypes, dtype=np.float32)

    x_sq = np.sum(x * x, axis=-1)  # [B]
    p_sq = np.sum(p * p, axis=-1)  # [P]

    psq_hi = p_sq.astype(bf16)
    psq_lo = (p_sq - psq_hi.astype(np.float32)).astype(bf16)
    augR = np.stack([psq_hi, psq_lo])  # [2, P]

    # [2, BT, P] with pT[k, r, n] = -2 * p[n, k*128 + r]
    pT = np.ascontiguousarray((-2.0 * p).T.reshape(2, BT, P)).astype(bf16)

    in_maps = []
    for c in range(N_CORES):
        xc = x[c * BS:(c + 1) * BS]  # [BS, F]
        xT = np.ascontiguousarray(xc.T.reshape(2, BT, BS)).astype(bf16)
        # xsq[r, bt] = x_sq[c*BS + bt*BT + r]
        xsq = np.ascontiguousarray(
            x_sq[c * BS:(c + 1) * BS].reshape(NB, BT).T).astype(np.float32)
        in_maps.append({"xT": xT, "pT": pT, "augR": augR, "xsq": xsq})
    return in_maps


def _gather(per_core_outs):
    """fp16 shards -> fp32 full output, undoing the exact 2^10 pre-scale."""
    out = np.concatenate(per_core_outs, axis=0).astype(np.float32)
    out *= np.float32(2.0 ** -OUT_SCALE_LOG2)
    return out


def kernel(x: np.ndarray, prototypes: np.ndarray) -> np.ndarray:
    nc = build_nc()
    in_maps = _prep_inputs(x, prototypes)
    res = run_bass_kernel_spmd(nc, in_maps, list(range(N_CORES)))
    return _gather([res.results[c]["out"] for c in range(N_CORES)])


# revision 5
# speedup vs baseline: 4201.7950x; 3.1721x over previous
"""Gaussian (norm) kernel matrix on 8 Trainium2 NeuronCores.

out[b, p] = exp(-sqrt(||x_b - proto_p||^2) / (2*sigma^2)), sigma = 1.

Sharding: x split along batch across 8 cores (1024 rows each); prototypes
replicated. Each core computes its [1024, 8192] slice.

Per-core pipeline (engines decoupled so ACT, the bottleneck, never waits):
- PE: psum = -2 x.p (two K=128 bf16 matmuls) + p_sq (K=2 bf16 aug matmul
  with hi/lo split rows against ones) into [128, 2048] PSUM chunks.
- DVE: d2 = fp16(psum + x_sq) - drains PSUM with the per-partition x_sq
  add folded in, freeing PSUM so the PE streams continuously.
- ACT: per phase (a group of b-tiles), sqrt in place over d2, then exp
  chunks written as fp16 with a 2^10 pre-scale (keeps the tiny outputs in
  fp16 normal range; the host divides it back out exactly). sqrt/exp live
  in different ACT table sets (~1.3us per load), so ACT instruction order
  is pinned with scheduling-only deps to get exactly 2 loads per phase.
  Phase sizes taper (4,2,1,1) so the final sqrt->exp->DMA tail after the
  last matmul is short.
"""

import numpy as np

import concourse.bacc as bacc
import concourse.mybir as mybir
import concourse.tile as tile
from concourse.tile import add_dep_helper
from concourse.bass_utils import run_bass_kernel_spmd

N_CORES = 8
B, P, F = 8192, 8192, 256
BS = B // N_CORES  # 1024 batch rows per core
BT = 128  # batch tile (partition dim)
NB = BS // BT  # 8 batch tiles per core
NCHUNK = 2048  # PSUM tile free size (4 banks; 2 bufs = all 8)
PHASES = (4, 4)  # b-tiles per activation-table phase
OCHUNK = 4096  # exp output chunk (free dim)
MMN = 512  # matmul moving free size
OUT_SCALE_LOG2 = 10  # exp pre-scale: out16 = 2^10 * exp(-dist/2)
F32 = mybir.dt.float32
F16 = mybir.dt.float16
BF16 = mybir.dt.bfloat16
AF = mybir.ActivationFunctionType


def build_nc(n_iters: int = 1, loop: bool = False):
    nc = bacc.Bacc("TRN2", target_bir_lowering=False, debug=False,
                   num_devices=N_CORES)
    xT_d = nc.dram_tensor("xT", [2, BT, BS], BF16, kind="ExternalInput")
    pT_d = nc.dram_tensor("pT", [2, BT, P], BF16, kind="ExternalInput")
    augR_d = nc.dram_tensor("augR", [2, P], BF16, kind="ExternalInput")
    xsq_d = nc.dram_tensor("xsq", [BT, NB], F32, kind="ExternalInput")
    out_d = nc.dram_tensor("out", [BS, P], F16, kind="ExternalOutput")

    with tile.TileContext(nc) as tc:
        with (
            tc.tile_pool(name="const", bufs=1) as cpool,
            tc.tile_pool(name="d2a", bufs=1) as d2pool_a,
            tc.tile_pool(name="d2b", bufs=1) as d2pool_b,
            tc.tile_pool(name="o16", bufs=3) as opool,
            tc.tile_pool(name="psum", bufs=2, space="PSUM") as ppool,
        ):
            xT = [cpool.tile([BT, BS], BF16, name=f"xT{k}", tag=f"xT{k}")
                  for k in range(2)]
            pT = [cpool.tile([BT, P], BF16, name=f"pT{k}", tag=f"pT{k}")
                  for k in range(2)]
            augL = cpool.tile([2, BS], BF16)
            augR = cpool.tile([2, P], BF16)
            xsq = cpool.tile([BT, NB], F32)
            ebias = cpool.tile([BT, 1], F32)
            nc.gpsimd.memset(ebias[:], float(OUT_SCALE_LOG2 * np.log(2.0)))
            nc.gpsimd.memset(augL[:], 1.0)
            for k in range(2):
                nc.sync.dma_start(xT[k][:], xT_d[k])
            nc.sync.dma_start(augR[:], augR_d[:])
            nc.sync.dma_start(xsq[:], xsq_d[:])
            for k in range(2):
                for c0 in range(0, P, NCHUNK):
                    nc.sync.dma_start(pT[k][:, c0:c0 + NCHUNK],
                                      pT_d[k, :, c0:c0 + NCHUNK])

            # consecutive phases alternate pools so DVE(k+1) overlaps ACT(k)
            d2pools = [d2pool_a, d2pool_b]

            acts = []  # pinned ACT execution order

            def body(_iv=None):
                bt0 = 0
                for pi, ph in enumerate(PHASES):
                    bts = range(bt0, bt0 + ph)
                    d2 = d2pools[pi % 2].tile([BT, ph * P], F16)
                    for bt in bts:
                        bsl = slice(bt * BT, (bt + 1) * BT)
                        xo = (bt - bt0) * P
                        for c0 in range(0, P, NCHUNK):
                            ps = ppool.tile([BT, NCHUNK], F32)
                            for j in range(0, NCHUNK, MMN):
                                n = slice(c0 + j, c0 + j + MMN)
                                o = ps[:, j:j + MMN]
                                nc.tensor.matmul(o, xT[0][:, bsl],
                                                 pT[0][:, n],
                                                 start=True, stop=False)
                                nc.tensor.matmul(o, xT[1][:, bsl],
                                                 pT[1][:, n],
                                                 start=False, stop=False)
                                nc.tensor.matmul(o, augL[:, bsl],
                                                 augR[:, n],
                                                 start=False, stop=True)
                            nc.vector.tensor_scalar_add(
                                d2[:, xo + c0:xo + c0 + NCHUNK], ps[:],
                                xsq[:, bt:bt + 1])
                    for bt in bts:
                        xo = (bt - bt0) * P
                        acts.append(nc.scalar.activation(
                            d2[:, xo:xo + P], d2[:, xo:xo + P], AF.Sqrt))
                    for k in range(0, ph * P, OCHUNK):
                        bt = bt0 + k // P
                        c0 = k % P
                        o16 = opool.tile([BT, OCHUNK], F16)
                        acts.append(nc.scalar.activation(
                            o16[:], d2[:, k:k + OCHUNK],
                            AF.Exp, scale=-0.5, bias=ebias[:]))
                        eng = nc.sync if (k // OCHUNK) % 2 == 0 else nc.scalar
                        eng.dma_start(
                            out_d[bt * BT:(bt + 1) * BT, c0:c0 + OCHUNK],
                            o16[:])
                    bt0 += ph

            if loop and n_iters > 1:
                with tc.For_i(0, n_iters, 1,
                              hint_engines=(mybir.EngineType.PE,
                                            mybir.EngineType.Activation,
                                            mybir.EngineType.DVE)):
                    body()
            else:
                for _ in range(n_iters):
                    body()
            for a, b in zip(acts, acts[1:]):
                add_dep_helper(b.ins, a.ins, sync=False,
                               reason="pin ACT order for table-set grouping")
    nc.compile()
    return nc


def _prep_inputs(x: np.ndarray, prototypes: np.ndarray):
    """Host-side shard + layout prep. Returns per-core in_maps."""
    bf16 = mybir.dt.np(BF16)
    x = np.ascontiguousarray(x, dtype=np.float32)
    p = np.ascontiguousarray(prototypes, dtype=np.float32)

    x_sq = np.sum(x * x, axis=-1)  # [B]
    p_sq = np.sum(p * p, axis=-1)  # [P]

    psq_hi = p_sq.astype(bf16)
    psq_lo = (p_sq - psq_hi.astype(np.float32)).astype(bf16)
    augR = np.stack([psq_hi, psq_lo])  # [2, P]

    # [2, BT, P] with pT[k, r, n] = -2 * p[n, k*128 + r]
    pT = np.ascontiguousarray((-2.0 * p).T.reshape(2, BT, P)).astype(bf16)

    in_maps = []
    for c in range(N_CORES):
        xc = x[c * BS:(c + 1) * BS]  # [BS, F]
        xT = np.ascontiguousarray(xc.T.reshape(2, BT, BS)).astype(bf16)
        # xsq[r, bt] = x_sq[c*BS + bt*BT + r]
        xsq = np.ascontiguousarray(
            x_sq[c * BS:(c + 1) * BS].reshape(NB, BT).T).astype(np.float32)
        in_maps.append({"xT": xT, "pT": pT, "augR": augR, "xsq": xsq})
    return in_maps


def _gather(per_core_outs):
    """fp16 shards -> fp32 full output, undoing the exact 2^10 pre-scale."""
    out = np.concatenate(per_core_outs, axis=0).astype(np.float32)
    out *= np.float32(2.0 ** -OUT_SCALE_LOG2)
    return out


def kernel(x: np.ndarray, prototypes: np.ndarray) -> np.ndarray:
    nc = build_nc()
    in_maps = _prep_inputs(x, prototypes)
    res = run_bass_kernel_spmd(nc, in_maps, list(range(N_CORES)))
    return _gather([res.results[c]["out"] for c in range(N_CORES)])


# revision 7
# speedup vs baseline: 4889.7354x; 1.1637x over previous
"""Gaussian (norm) kernel matrix on 8 Trainium2 NeuronCores - fused ACT.

out[b, p] = exp(-sqrt(||x_b - proto_p||^2) / (2*sigma^2)), sigma = 1.

Sharding: x split along batch across 8 cores (1024 rows each); prototypes
replicated. Each core computes its [1024, 8192] slice.

The scalar engine evaluates activations from loadable piecewise-cubic
spline tables (bucket coefficients indexed by exponent/mantissa). We refit
the bucket coefficients of the stock `sqrt` entry to

    g(t) = 2^10 * exp(-sqrt(t)/2)

(least-squares cubic per bucket interval) and point the compiler at the
patched table directory via BASS_ACT_ROOT_JSON_PATH. The kernel's AF.Sqrt
instruction then computes the whole distance->kernel map in ONE pass
directly from PSUM, with the per-row ||x||^2 folded in via the ACT
per-partition bias:

- PE: psum = -2 x.p (two K=128 bf16 matmuls only).
- DVE: d2 = (psum + x_sq) + p_sq_rep (scalar_tensor_tensor) into SBUF f32
  chunks, draining PSUM so the PE streams continuously; p_sq arrives
  pre-replicated across partitions as a [128, P] f32 constant.
- ACT: o16 = g(d2) as fp16 (2^10 pre-scale keeps the tiny outputs in fp16
  normal range; the host divides it back out exactly).
- One ACT pass, no table switches: the three engines pipeline per chunk.
"""

import json
import os
import shutil
import tempfile
from pathlib import Path

import numpy as np

import concourse.bacc as bacc
import concourse.mybir as mybir
import concourse.tile as tile
from concourse.bass_utils import run_bass_kernel_spmd

N_CORES = 8
B, P, F = 8192, 8192, 256
BS = B // N_CORES  # 1024 batch rows per core
BT = 128  # batch tile (partition dim)
NB = BS // BT  # 8 batch tiles per core
NCHUNK = 2048  # PSUM tile free size (4 banks; 2 bufs = all 8)
MMN = 512  # matmul moving free size
OUT_SCALE_LOG2 = 10  # table pre-scale: out16 = 2^10 * exp(-dist/2)
ACT_TABLE_VERSION = 1  # bump to bust the NEFF cache when tables change
F32 = mybir.dt.float32
F16 = mybir.dt.float16
BF16 = mybir.dt.bfloat16
AF = mybir.ActivationFunctionType

_ACT_DIR = None


def _g(t):
    """Target activation: 2^10 * exp(-sqrt(t)/2), in float64."""
    t = np.asarray(t, dtype=np.float64)
    with np.errstate(over="ignore", under="ignore"):
        return np.exp2(OUT_SCALE_LOG2) * np.exp(-np.sqrt(np.maximum(t, 0.0)) / 2.0)


def _patch_bkt(path: Path):
    """Refit every sqrt bucket in a *_bkt.bin to g (LSQ cubic per bucket).

    Bucket record: 8 fp32 = [d0, d1, d2, d3, x0, 0, 0, 0] with
    f(x) ~= d0 + d1 (x-x0) + d2 (x-x0)^2 + d3 (x-x0)^3.
    sqrt buckets are identified by d0 ~= sqrt(x0) AND d1 ~= 0.5/sqrt(x0).
    """
    b = np.fromfile(path, dtype=np.float32).reshape(-1, 8)
    x0 = b[:, 4].astype(np.float64)
    d0 = b[:, 0].astype(np.float64)
    d1 = b[:, 1].astype(np.float64)
    with np.errstate(invalid="ignore", divide="ignore"):
        s = np.sqrt(x0)
        is_sqrt = (
            (x0 > 0) & np.isfinite(x0)
            & (np.abs(d0 - s) <= 1e-3 * np.abs(s))
            & (np.abs(d1 - 0.5 / s) <= 1e-3 * np.abs(0.5 / s))
        )
    idx = np.where(is_sqrt)[0]
    assert idx.size > 500, f"only {idx.size} sqrt buckets found in {path}"
    xs = x0[idx]
    order = np.argsort(xs)
    sx = xs[order]
    gaps = np.diff(sx)
    half = np.empty_like(sx)
    half[1:-1] = np.minimum(gaps[:-1], gaps[1:]) / 2.0
    half[0] = gaps[0] / 2.0
    half[-1] = gaps[-1] / 2.0

    u = np.linspace(-1.0, 1.0, 33)
    f32max = np.float64(3.0e38)
    for k, oi in enumerate(order):
        i = idx[oi]
        c, h = sx[k], half[k]
        ts = c + h * u
        ys = _g(ts)
        if not np.all(np.isfinite(ys)):
            ys = np.where(np.isfinite(ys), ys, 0.0)
        if np.all(ys == 0.0):
            co = np.zeros(4)
        else:
            # fit on the scaled basis u = (x-c)/h, then unscale; zero any
            # term that overflows f32 (only happens for x0 far outside the
            # kernel's input range, where the term's in-interval
            # contribution is negligible anyway)
            a = np.polynomial.polynomial.polyfit(u, ys, 3)
            co = a / np.power(h, np.arange(4.0))
            co = np.where(np.isfinite(co) & (np.abs(co) < f32max), co, 0.0)
        b[i, 0:4] = co.astype(np.float32)
    b.tofile(path)


def _ensure_act_tables():
    """Write a patched act-table dir and point the compiler at it."""
    global _ACT_DIR
    if _ACT_DIR is not None:
        return
    from neuronxcc.driver.Job import Job
    from neuronxcc.driver.jobs.support.FindActInfo import findActInfoFile

    src = Path(findActInfoFile(Job.getPackageDir(), "gen3")).parent
    tmp = Path(tempfile.mkdtemp(prefix="act_fused_"))
    for f in src.iterdir():
        if f.is_file():
            shutil.copy(f, tmp / f.name)
    with open(tmp / "act_info.json") as fh:
        info = json.load(fh)
    n = 0
    for ent in info["act_func_sets"]:
        if "sqrt" in ent["act"]:
            _patch_bkt(tmp / ent["bkt_bin"])
            n += 1
    assert n >= 1, "no sqrt table set found to patch"
    os.environ["BASS_ACT_ROOT_JSON_PATH"] = str(tmp / "act_info.json")
    _ACT_DIR = tmp


def build_nc(n_iters: int = 1, loop: bool = False):
    _ensure_act_tables()
    nc = bacc.Bacc("TRN2", target_bir_lowering=False, debug=False,
                   num_devices=N_CORES)
    xT_d = nc.dram_tensor("xT", [2, BT, BS], BF16, kind="ExternalInput")
    pT_d = nc.dram_tensor("pT", [2, BT, P], BF16, kind="ExternalInput")
    psq_d = nc.dram_tensor("psq", [BT, P], F16, kind="ExternalInput")
    xsq_d = nc.dram_tensor("xsq", [BT, NB], F32, kind="ExternalInput")
    out_d = nc.dram_tensor("out", [BS, P], F16, kind="ExternalOutput")

    with tile.TileContext(nc) as tc:
        with (
            tc.tile_pool(name="const", bufs=1) as cpool,
            tc.tile_pool(name="o16", bufs=8) as opool,
            tc.tile_pool(name="d2c", bufs=6) as dpool,
            tc.tile_pool(name="psum", bufs=2, space="PSUM") as ppool,
        ):
            xT = [cpool.tile([BT, BS], BF16, name=f"xT{k}", tag=f"xT{k}")
                  for k in range(2)]
            pT = [cpool.tile([BT, P], BF16, name=f"pT{k}", tag=f"pT{k}")
                  for k in range(2)]
            psq = cpool.tile([BT, P], F16)
            xsq = cpool.tile([BT, NB], F32)
            # cache-buster: table payload isn't in the NEFF hash
            ver = cpool.tile([1, 2], F32, name=f"actver{ACT_TABLE_VERSION}")
            nc.gpsimd.memset(ver[:], float(ACT_TABLE_VERSION))
            for k in range(2):
                nc.sync.dma_start(xT[k][:], xT_d[k])
            nc.sync.dma_start(xsq[:], xsq_d[:])
            for c0 in range(0, P, NCHUNK):
                nc.scalar.dma_start(psq[:, c0:c0 + NCHUNK],
                                    psq_d[:, c0:c0 + NCHUNK])
            for k in range(2):
                for c0 in range(0, P, NCHUNK):
                    nc.sync.dma_start(pT[k][:, c0:c0 + NCHUNK],
                                      pT_d[k, :, c0:c0 + NCHUNK])

            def body(_iv=None):
                for bt in range(NB):
                    bsl = slice(bt * BT, (bt + 1) * BT)
                    for c0 in range(0, P, NCHUNK):
                        ps = ppool.tile([BT, NCHUNK], F32)
                        # k-outer order: one LDWEIGHTS per stationary per
                        # chunk instead of one per matmul
                        for k in range(2):
                            for j in range(0, NCHUNK, MMN):
                                n = slice(c0 + j, c0 + j + MMN)
                                nc.tensor.matmul(ps[:, j:j + MMN],
                                                 xT[k][:, bsl], pT[k][:, n],
                                                 start=(k == 0),
                                                 stop=(k == 1))
                        d2c = dpool.tile([BT, NCHUNK], F32)
                        nc.vector.scalar_tensor_tensor(
                            d2c[:], ps[:], xsq[:, bt:bt + 1],
                            psq[:, c0:c0 + NCHUNK],
                            mybir.AluOpType.add, mybir.AluOpType.add)
                        o16 = opool.tile([BT, NCHUNK], F16)
                        nc.scalar.activation(o16[:], d2c[:], AF.Sqrt)
                        eng = nc.sync if (c0 // NCHUNK) % 2 == 0 else nc.scalar
                        eng.dma_start(
                            out_d[bt * BT:(bt + 1) * BT, c0:c0 + NCHUNK],
                            o16[:])

            if loop and n_iters > 1:
                with tc.For_i(0, n_iters, 1,
                              hint_engines=(mybir.EngineType.PE,
                                            mybir.EngineType.Activation)):
                    body()
            else:
                for _ in range(n_iters):
                    body()
    nc.compile()
    return nc


def _prep_inputs(x: np.ndarray, prototypes: np.ndarray):
    """Host-side shard + layout prep. Returns per-core in_maps."""
    bf16 = mybir.dt.np(BF16)
    x = np.ascontiguousarray(x, dtype=np.float32)
    p = np.ascontiguousarray(prototypes, dtype=np.float32)

    x_sq = np.sum(x * x, axis=-1)  # [B]
    p_sq = np.sum(p * p, axis=-1)  # [P]

    psq_rep = np.ascontiguousarray(
        np.broadcast_to(p_sq[None, :], (BT, P))).astype(np.float16)

    # [2, BT, P] with pT[k, r, n] = -2 * p[n, k*128 + r]
    pT = np.ascontiguousarray((-2.0 * p).T.reshape(2, BT, P)).astype(bf16)

    in_maps = []
    for c in range(N_CORES):
        xc = x[c * BS:(c + 1) * BS]  # [BS, F]
        xT = np.ascontiguousarray(xc.T.reshape(2, BT, BS)).astype(bf16)
        # xsq[r, bt] = x_sq[c*BS + bt*BT + r]
        xsq = np.ascontiguousarray(
            x_sq[c * BS:(c + 1) * BS].reshape(NB, BT).T).astype(np.float32)
        in_maps.append({"xT": xT, "pT": pT, "psq": psq_rep, "xsq": xsq})
    return in_maps


def _gather(per_core_outs):
    """fp16 shards -> fp32 full output, undoing the exact 2^10 pre-scale."""
    out = np.concatenate(per_core_outs, axis=0).astype(np.float32)
    out *= np.float32(2.0 ** -OUT_SCALE_LOG2)
    return out


def kernel(x: np.ndarray, prototypes: np.ndarray) -> np.ndarray:
    nc = build_nc()
    in_maps = _prep_inputs(x, prototypes)
    res = run_bass_kernel_spmd(nc, in_maps, list(range(N_CORES)))
    return _gather([res.results[c]["out"] for c in range(N_CORES)])
